# revision 43
# baseline (speedup 1.0000x reference)
"""3-layer GAT on 8 Trainium2 NeuronCores.

Strategy (edge-parallel, dst-sharded), v3:
  - The bottleneck is SWDGE descriptor generation: every 128-row
    indirect gather costs ~994ns of fixed GPSIMD-engine time, and each
    of the 3 layers needs E/8/128 = 784 gathers per core (~870us/layer
    on the Pool engine).  Everything else is organized to hide under
    that serial gather stream.
  - Nodes relabeled so each core owns bpc bins of 128 destination slots,
    balanced by in-degree. Self-loop edges are NOT materialized: their
    contribution is folded in at flush time (elementwise), which drops
    one edge-chunk per bin (SWDGE calls are the bottleneck: each
    128-row indirect gather costs ~1us of fixed descriptor-gen time on
    the GPSIMD engine regardless of payload).
  - The gathered table is bf16 and carries only [h | a_src] (520 cols
    for layers 0/1, 65 for layer 2). a_dst stays in SBUF on the owning
    core. Pad edge slots point at slot 128 so their one-hot column is
    all-zero: no sentinel row and no contribution to either the
    numerator or the denominator.
  - The table rows are laid out group-major (NAG groups of tiles x 8
    cores) so the AllGather can be issued as NAG chunked collectives,
    each fired as soon as its tile-group's node phase is done. Node
    phase / flush / AllGather issue is interleaved into the previous
    layer's aggregation loop so collectives and PE/DVE work hide under
    the serial SWDGE gather stream.
  - Activations between layers live in SBUF (no DRAM round-trip); the
    input x' and the per-chunk gather indices are loaded once.
  - Layer 2's log-softmax Ln is batched into one pass at the end (one
    activation-table load instead of 2 per tile).
"""

import sys

try:
    import concourse  # noqa: F401  (provided via PYTHONPATH on axon hosts)
except ImportError:
    sys.path.insert(0, "/opt/trn_rl_repo")

import heapq

import numpy as np
import ml_dtypes

import concourse.bacc as bacc
import concourse.bass as bass
import concourse.mybir as mybir
import concourse.tile as tile

P = 128
NCORES = 8
NH = 8          # heads (layers 0/1)
HC = 64         # channels per head
HD = NH * HC    # 512
D0 = 128        # input feature dim
OUTC = 40       # final classes
CO = 64         # padded layer-2 width
COL0 = HD + NH  # 520 packed row: h(512) | a_src(8)
COL2 = CO + 1   # 65  packed row: h2(64) | a_src2
G = 8           # chunks per gather super-chunk
NAG = 2         # allgather chunks per layer
AG0_FRAC = 0.41  # fraction of tiles in the first allgather group
NEG = 0.2       # leaky relu slope

f32 = mybir.dt.float32
bf16 = mybir.dt.bfloat16
i32 = mybir.dt.int32
AF = mybir.ActivationFunctionType
ALU = mybir.AluOpType
BF = ml_dtypes.bfloat16


# ----------------------------------------------------------------- host prep

def _balance_bins(deg, nbins):
    """Assign each node to a 128-slot bin, balancing summed in-degree."""
    n = deg.shape[0]
    order = np.argsort(-deg, kind="stable")
    bin_of = np.empty(n, np.int32)
    slot_of = np.empty(n, np.int32)
    counts = np.zeros(nbins, np.int32)
    loads = np.zeros(nbins, np.int64)
    heap = [(0, b) for b in range(nbins)]
    heapq.heapify(heap)
    for node in order:
        while True:
            _, b = heapq.heappop(heap)
            if counts[b] < P:
                break
        bin_of[node] = b
        slot_of[node] = counts[b]
        counts[b] += 1
        loads[b] += deg[node]
        if counts[b] < P:
            heapq.heappush(heap, (int(loads[b]), b))
    return bin_of, slot_of, loads


def _groups(bpc):
    """Split bpc tiles into NAG groups; group 0 is smaller so its
    allgather fires (and lands) earlier in the previous layer's tail."""
    g0 = max(1, int(bpc * AG0_FRAC))
    return [(0, g0), (g0, bpc)]


def _prep(edge_index, n_nodes):
    src = np.asarray(edge_index[0], dtype=np.int64)
    dst = np.asarray(edge_index[1], dtype=np.int64)
    # self-loops are folded in analytically at flush time; only the real
    # edges are materialized.
    deg = np.bincount(dst, minlength=n_nodes)
    bpc = -(-n_nodes // (P * NCORES))          # bins per core
    nbins = bpc * NCORES
    npad = nbins * P
    nloc = bpc * P

    bin_of, slot_of, loads = _balance_bins(deg, nbins)
    new_id = bin_of.astype(np.int64) * P + slot_of

    cpt = int(-(-int(loads.max()) // P))        # chunks per tile
    ch = bpc * cpt                              # chunks per core
    nsup = -(-ch // G)

    # group-major table row permutation: tabrow[new_id] -> row in hcat
    grp = _groups(bpc)
    tabrow = np.empty(npad, np.int64)
    slab_off = 0
    for (t0, t1) in grp:
        rbj = (t1 - t0) * P
        for c in range(NCORES):
            for tt in range(t0, t1):
                b = c * bpc + tt
                rows = np.arange(b * P, (b + 1) * P)
                tabrow[rows] = slab_off + c * rbj + (tt - t0) * P + \
                    np.arange(P)
        slab_off += NCORES * rbj
    slab0 = NCORES * (grp[0][1] - grp[0][0]) * P

    e_src = tabrow[new_id[src]].astype(np.int32)
    e_dst = new_id[dst].astype(np.int32)
    e_bin = (e_dst >> 7).astype(np.int64)
    e_slot = (e_dst & 127).astype(np.float32)

    order_e = np.argsort(e_bin, kind="stable")
    starts = np.zeros(nbins + 1, np.int64)
    starts[1:] = np.cumsum(np.bincount(e_bin, minlength=nbins))

    ept = cpt * P
    src_arr = np.zeros((NCORES, ch, P), np.int32)       # pad src -> row 0
    dstf_arr = np.full((NCORES, ch, P), float(P), np.float32)  # pad slot 128
    n0_arr = np.zeros((NCORES, bpc), np.int64)
    for b in range(nbins):
        es = order_e[starts[b]:starts[b + 1]]
        c, t = divmod(b, bpc)
        # sources in allgather group 0 first, so early chunks only touch
        # the first table slab and can gather while AG_1 is in flight
        is0 = e_src[es] < slab0
        es = np.concatenate([es[is0], es[~is0]])
        n0_arr[c, t] = int(is0.sum())
        pos = t * ept + np.arange(es.shape[0])
        chunk = pos >> 7
        pp = pos & 127
        src_arr[c, chunk, pp] = e_src[es]
        dstf_arr[c, chunk, pp] = e_slot[es]

    # chunks (per tile) guaranteed to touch only slab 0 on EVERY core,
    # capped: the early phase only needs to cover the AG_1 flight time
    nearly = tuple(min(6, int(x)) for x in (n0_arr.min(axis=0) // P))

    # two-phase processing order: every tile's early (slab-0-only) chunks
    # first, then all late chunks -> the gather queue never stalls on the
    # second allgather chunk
    sched = [(t, c) for t in range(bpc) for c in range(nearly[t])] + \
            [(t, c) for t in range(bpc) for c in range(nearly[t], cpt)]
    order = np.array([t * cpt + c for (t, c) in sched], np.int64)

    per_core = []
    for c in range(NCORES):
        per_core.append({
            # [P, ch] so a single DMA loads all chunk columns
            "srcidx": np.ascontiguousarray(src_arr[c][order].T),
            "dstf": np.ascontiguousarray(dstf_arr[c][order].T).astype(BF),
        })

    dims = dict(n=n_nodes, bpc=bpc, nbins=nbins, npad=npad, nloc=nloc,
                cpt=cpt, ch=ch, nsup=nsup, slab0=slab0, nearly=nearly)
    return dims, per_core, new_id


def _block_diag_a(att_s, att_d):
    """[NH,HC]x2 -> [HD, 2*NH] block matrix for a = h @ A."""
    a = np.zeros((HD, 2 * NH), np.float32)
    r = np.arange(HD)
    h = r >> 6
    c = r & 63
    a[r, h] = att_s[h, c]
    a[r, NH + h] = att_d[h, c]
    return a


# ------------------------------------------------------------- device build

def _build(dims, single=False, shared_ag=True):
    npad, nloc, bpc = dims["npad"], dims["nloc"], dims["bpc"]
    cpt, ch, nsup = dims["cpt"], dims["ch"], dims["nsup"]
    slab0, nearly = dims["slab0"], dims["nearly"]
    grp = _groups(bpc)

    nc = bacc.Bacc("TRN2", target_bir_lowering=False, debug=False,
                   enable_asserts=True,
                   num_devices=1 if single else NCORES)

    # inputs
    xt_ap = nc.dram_tensor("xt", [D0, nloc], bf16, kind="ExternalInput").ap()
    srcidx_ap = nc.dram_tensor("srcidx", [P, ch], i32,
                               kind="ExternalInput").ap()
    dstf_ap = nc.dram_tensor("dstf", [P, ch], bf16, kind="ExternalInput").ap()
    w0h_ap = nc.dram_tensor("w0h", [D0, HD], bf16, kind="ExternalInput").ap()
    w0a_ap = nc.dram_tensor("w0a", [D0, 2 * NH], bf16,
                            kind="ExternalInput").ap()
    w1h_ap = nc.dram_tensor("w1h", [HD, HD], bf16, kind="ExternalInput").ap()
    w1a_ap = nc.dram_tensor("w1a", [HD, 2 * NH], bf16,
                            kind="ExternalInput").ap()
    w2e_ap = nc.dram_tensor("w2e", [HD, COL2 + 1], bf16,
                            kind="ExternalInput").ap()
    b0_ap = nc.dram_tensor("b0r", [1, HD], f32, kind="ExternalInput").ap()
    b1_ap = nc.dram_tensor("b1r", [1, HD], f32, kind="ExternalInput").ap()
    b2_ap = nc.dram_tensor("b2r", [1, CO], f32, kind="ExternalInput").ap()
    iota_ap = nc.dram_tensor("iotag", [P, P * G], bf16,
                             kind="ExternalInput").ap()
    ident_ap = nc.dram_tensor("identb", [P, P], bf16,
                              kind="ExternalInput").ap()
    out_ap = nc.dram_tensor("out", [nloc, OUTC], f32,
                            kind="ExternalOutput").ap()

    hcat = []
    for l, col in ((0, COL0), (1, COL0), (2, COL2)):
        if single or not shared_ag:
            t = nc.dram_tensor(f"hcat{l}", [npad, col], bf16, kind="Internal")
        else:
            t = nc.dram_tensor(f"hcat{l}", [npad, col], bf16,
                               kind="Internal", addr_space="Shared")
        hcat.append(t.ap())

    with tile.TileContext(nc) as tc:
        with tc.tile_pool(name="const", bufs=1) as cp, \
             tc.tile_pool(name="work", bufs=3) as sb, \
             tc.tile_pool(name="psum", bufs=2, space="PSUM") as ps, \
             tc.tile_pool(name="dram", bufs=1, space="DRAM") as dp:

            # ---------- persistent constants / state in SBUF
            # iotag[p, j*G+g] = j  (slot-major so the one-hot build keeps
            # every operand's last AP dim packed -> DVE 2x mode)
            iota_t = cp.tile([P, P * G], bf16)
            ident_t = cp.tile([P, P], bf16)
            ones_t = cp.tile([1, P], f32)

            xt_sb = cp.tile([P, nloc], bf16)
            g0p = grp[0][1] * P
            nc.sync.dma_start(xt_sb[:, :g0p], xt_ap[:, :g0p])
            nc.sync.dma_start(xt_sb[:, g0p:], xt_ap[:, g0p:])
            w0h_t = cp.tile([P, HD], bf16)
            nc.sync.dma_start(w0h_t[:], w0h_ap[:])
            w0a_t = cp.tile([P, 2 * NH], bf16)
            nc.sync.dma_start(w0a_t[:], w0a_ap[:])
            actt = cp.tile([P, 4 * nloc], bf16)   # transposed activations

            idx_t = cp.tile([P, ch], i32)
            dstf_t = cp.tile([P, ch], bf16)
            w1h_t = [cp.tile([P, HD], bf16, name=f"w1h{k}", tag=f"w1h_{k}")
                     for k in range(4)]
            w1a_t = [cp.tile([P, 2 * NH], bf16, name=f"w1a{k}", tag=f"w1a_{k}")
                     for k in range(4)]
            w2e_t = [cp.tile([P, COL2 + 1], bf16, name=f"w2e{k}",
                             tag=f"w2e_{k}") for k in range(4)]

            b0_t = cp.tile([P, HD], bf16, name="b0t", tag="b0t")
            b1_t = cp.tile([P, HD], bf16, name="b1t", tag="b1t")
            b2_t = cp.tile([P, CO], bf16, name="b2t", tag="b2t")

            def fill_bias(b_ap, width, bt):
                row = sb.tile([1, width], f32, tag="brow")
                nc.sync.dma_start(row[:], b_ap[:])
                bps = ps.tile([P, width], f32, tag="agg", bufs=4)
                nc.tensor.matmul(out=bps[:], lhsT=ones_t[:], rhs=row[:],
                                 start=True, stop=True)
                nc.scalar.activation(bt[:], bps[:], AF.Copy)

            def load_late_consts():
                # issued after the first AG group: the startup HWDGE/PE
                # queues only carry what layer 0's node phase needs
                nc.sync.dma_start(idx_t[:], srcidx_ap[:])
                nc.sync.dma_start(dstf_t[:], dstf_ap[:])
                nc.sync.dma_start(iota_t[:], iota_ap[:])
                nc.sync.dma_start(ident_t[:], ident_ap[:])
                nc.gpsimd.memset(ones_t[:], 1.0)
                for k in range(4):
                    nc.sync.dma_start(w1h_t[k][:],
                                      w1h_ap[k * P:(k + 1) * P, :])
                    nc.sync.dma_start(w1a_t[k][:],
                                      w1a_ap[k * P:(k + 1) * P, :])
                    nc.sync.dma_start(w2e_t[k][:],
                                      w2e_ap[k * P:(k + 1) * P, :])
                fill_bias(b0_ap, HD, b0_t)
                fill_bias(b1_ap, HD, b1_t)
                fill_bias(b2_ap, CO, b2_t)

            # per-layer a_src/a_dst of own nodes (SBUF resident)
            # combined per-tile [a_src(8) | a_dst(8)] so the node phase
            # stores both with a single copy
            aad_sb = [cp.tile([P, bpc * 2 * NH], bf16, name=f"aad{l}",
                              tag=f"aad{l}") for l in range(2)]
            aad2_sb = cp.tile([P, bpc * 2], bf16)
            t2_all = cp.tile([P, bpc * OUTC], f32)
            sm_all = cp.tile([P, bpc], f32)

            # ---------- DRAM scratch
            opart = dp.tile([nloc, HD], bf16, name="opart")
            dpart = cp.tile([P, bpc * NH], f32)
            ag_in = [dp.tile([nloc, COL0], bf16, name="ag0"),
                     dp.tile([nloc, COL0], bf16, name="ag1"),
                     dp.tile([nloc, COL2], bf16, name="ag2")]

            # ---------- helpers
            def allgather_group(l, j):
                col = COL2 if l == 2 else COL0
                (t0, t1) = grp[j]
                rbj = (t1 - t0) * P
                so = sum((g1 - g0) * P * NCORES for (g0, g1) in grp[:j])
                src_rows = ag_in[l][t0 * P:t1 * P, :]
                dst_rows = hcat[l][so:so + NCORES * rbj, :]
                if single:
                    # chunked so the copy interleaves with gather-row DMAs
                    # on the (model-serialized) DMA engines
                    for r0 in range(0, rbj, 1024):
                        r1 = min(r0 + 1024, rbj)
                        nc.sync.dma_start(hcat[l][so + r0:so + r1, :],
                                          ag_in[l][t0 * P + r0:t0 * P + r1, :])
                else:
                    nc.gpsimd.collective_compute(
                        "AllGather", ALU.bypass,
                        replica_groups=[list(range(NCORES))],
                        ins=[src_rows.opt()],
                        outs=[dst_rows.opt()],
                    )

            # rolling 4-tile staging for the ag_in writes (1 DMA / 4 tiles)
            catstate = {}
            grp_ends = {t1 for (_t0, t1) in grp}

            def node_tile(l, t):
                """h/a projections for tile t of layer l; stages into a
                4-tile cat buffer (flushed to ag_in[l] on the 4th tile or a
                group boundary), plus asrc/adst SBUF tiles."""
                if l == 0:
                    h_ps = ps.tile([P, HD], f32, tag="agg", bufs=4)
                    a_ps = ps.tile([P, 2 * NH], f32, tag="den", bufs=2)
                    lhs = xt_sb[:, t * P:(t + 1) * P]
                    nc.tensor.matmul(out=h_ps[:], lhsT=lhs, rhs=w0h_t[:],
                                     start=True, stop=True)
                    nc.tensor.matmul(out=a_ps[:], lhsT=lhs, rhs=w0a_t[:],
                                     start=True, stop=True)
                elif l == 1:
                    h_ps = ps.tile([P, HD], f32, tag="agg", bufs=4)
                    a_ps = ps.tile([P, 2 * NH], f32, tag="den", bufs=2)
                    for k in range(4):
                        lhs = actt[:, k * nloc + t * P:k * nloc + (t + 1) * P]
                        nc.tensor.matmul(out=h_ps[:], lhsT=lhs,
                                         rhs=w1h_t[k][:],
                                         start=(k == 0), stop=(k == 3))
                        nc.tensor.matmul(out=a_ps[:], lhsT=lhs,
                                         rhs=w1a_t[k][:],
                                         start=(k == 0), stop=(k == 3))
                else:
                    h_ps = ps.tile([P, COL2 + 1], f32, tag="den", bufs=2)
                    for k in range(4):
                        lhs = actt[:, k * nloc + t * P:k * nloc + (t + 1) * P]
                        nc.tensor.matmul(out=h_ps[:], lhsT=lhs,
                                         rhs=w2e_t[k][:],
                                         start=(k == 0), stop=(k == 3))

                colw = COL0 if l < 2 else COL2
                if l not in catstate:
                    catstate[l] = (t, sb.tile([P, 4 * colw], bf16,
                                              name="cat4", tag="cat4"))
                t0b, cat = catstate[l]
                off = (t - t0b) * colw
                if l < 2:
                    # alternate engines so the node-phase copy stream is not
                    # serialized on Activation alone (startup critical path)
                    if t % 2 == 0:
                        nc.scalar.activation(cat[:, off:off + HD], h_ps[:],
                                             AF.Copy)
                    else:
                        nc.vector.tensor_copy(cat[:, off:off + HD], h_ps[:])
                    nc.vector.tensor_copy(cat[:, off + HD:off + COL0],
                                          a_ps[:, :NH])
                    nc.vector.tensor_copy(
                        aad_sb[l][:, t * 2 * NH:(t + 1) * 2 * NH], a_ps[:])
                else:
                    nc.scalar.activation(cat[:, off:off + COL2],
                                         h_ps[:, :COL2], AF.Copy)
                    nc.vector.tensor_copy(aad2_sb[:, 2 * t:2 * t + 2],
                                          h_ps[:, CO:CO + 2])
                nb = t - t0b + 1
                if nb == 4 or (t + 1) in grp_ends:
                    nc.sync.dma_start(
                        ag_in[l][t0b * P:(t + 1) * P, :]
                        .rearrange("(b p) c -> p b c", p=P),
                        cat[:, :nb * colw]
                        .rearrange("p (b c) -> p b c", c=colw))
                    del catstate[l]

            def self_ex(l, t, nhh):
                """exp(leaky_relu(asrc_own + adst_own)) for tile t: [P,nhh]."""
                e = sb.tile([P, NH], f32, tag="se")
                if l == 2:
                    nc.vector.tensor_tensor(out=e[:, :1],
                                            in0=aad2_sb[:, 2 * t:2 * t + 1],
                                            in1=aad2_sb[:, 2 * t + 1:2 * t + 2],
                                            op=ALU.add)
                else:
                    nc.vector.tensor_tensor(
                        out=e[:, :nhh],
                        in0=aad_sb[l][:, t * 2 * NH:t * 2 * NH + NH],
                        in1=aad_sb[l][:, t * 2 * NH + NH:(t + 1) * 2 * NH],
                        op=ALU.add)
                tmp = sb.tile([P, NH], f32, tag="se2")
                nc.vector.tensor_scalar_mul(tmp[:, :nhh], e[:, :nhh], NEG)
                nc.vector.tensor_tensor(out=e[:, :nhh], in0=e[:, :nhh],
                                        in1=tmp[:, :nhh], op=ALU.max)
                exs = sb.tile([P, NH], f32, tag="se3")
                nc.scalar.activation(exs[:, :nhh], e[:, :nhh], AF.Exp)
                return exs

            def flush_big(l, t, o_ps, d_ps, b_t, hp, dsl):
                """Normalize tile t of layer l (0/1), add self-loop + early
                partial, bias, relu, transpose into actt."""
                exs = self_ex(l, t, NH)
                hown = sb.tile([P, HD], bf16, tag="hown")
                nc.sync.dma_start(hown[:], ag_in[l][t * P:(t + 1) * P, :HD])
                den = sb.tile([P, NH], f32, tag="den_sb")
                nc.vector.tensor_tensor(out=den[:], in0=d_ps[:],
                                        in1=exs[:, :NH], op=ALU.add)
                if dsl is not None:
                    nc.vector.tensor_tensor(out=den[:], in0=den[:],
                                            in1=dsl, op=ALU.add)
                rden = sb.tile([P, NH], f32, tag="rden")
                nc.vector.reciprocal(rden[:], den[:])
                o_sb = sb.tile([P, HD], f32, tag="osb")
                # o = o_ps + exs*h_own  (self-loop contribution);
                # h columns are (c,h)-interleaved
                nc.vector.tensor_tensor(
                    out=o_sb[:].rearrange("p (c h) -> p c h", h=NH),
                    in0=hown[:].rearrange("p (c h) -> p c h", h=NH),
                    in1=exs[:, :NH].rearrange("p (one h) -> p one h", one=1)
                        .broadcast_to([P, HC, NH]),
                    op=ALU.mult)
                nc.vector.tensor_tensor(out=o_sb[:], in0=o_sb[:], in1=o_ps[:],
                                        op=ALU.add)
                if hp is not None:
                    nc.vector.tensor_tensor(out=o_sb[:], in0=o_sb[:],
                                            in1=hp[:], op=ALU.add)
                o2 = sb.tile([P, HD], bf16, tag="osb2")
                nc.vector.tensor_tensor(
                    out=o2[:].rearrange("p (c h) -> p c h", h=NH),
                    in0=o_sb[:].rearrange("p (c h) -> p c h", h=NH),
                    in1=rden[:].rearrange("p (one h) -> p one h", one=1)
                        .broadcast_to([P, HC, NH]),
                    op=ALU.mult)
                nc.vector.tensor_tensor(out=o2[:], in0=o2[:], in1=b_t[:],
                                        op=ALU.add)
                nc.scalar.activation(o2[:], o2[:], AF.Relu)
                for k in range(4):
                    tr_ps = ps.tile([P, P], bf16, tag="tr", bufs=1)
                    nc.tensor.transpose(out=tr_ps[:],
                                        in_=o2[:, k * P:(k + 1) * P],
                                        identity=ident_t[:])
                    nc.scalar.activation(
                        actt[:, k * nloc + t * P:k * nloc + (t + 1) * P],
                        tr_ps[:], AF.Copy)

            def flush_l2(t, o_ps, d_ps, hp, dsl):
                exs = self_ex(2, t, 1)
                hown = sb.tile([P, CO], bf16, tag="hown")
                nc.sync.dma_start(hown[:], ag_in[2][t * P:(t + 1) * P, :CO])
                den = sb.tile([P, 1], f32, tag="den_sb")
                nc.vector.tensor_tensor(out=den[:], in0=d_ps[:],
                                        in1=exs[:, :1], op=ALU.add)
                if dsl is not None:
                    nc.vector.tensor_tensor(out=den[:], in0=den[:],
                                            in1=dsl, op=ALU.add)
                rden = sb.tile([P, 1], f32, tag="rden")
                nc.vector.reciprocal(rden[:], den[:])
                o_sb = sb.tile([P, CO], f32, tag="osb")
                nc.vector.tensor_tensor(out=o_sb[:], in0=hown[:],
                                        in1=exs[:, :1].broadcast_to([P, CO]),
                                        op=ALU.mult)
                nc.vector.tensor_tensor(out=o_sb[:], in0=o_sb[:], in1=o_ps[:],
                                        op=ALU.add)
                if hp is not None:
                    nc.vector.tensor_tensor(out=o_sb[:], in0=o_sb[:],
                                            in1=hp[:, :CO], op=ALU.add)
                nc.vector.tensor_tensor(out=o_sb[:], in0=o_sb[:],
                                        in1=rden[:].broadcast_to([P, CO]),
                                        op=ALU.mult)
                nc.vector.tensor_tensor(out=o_sb[:], in0=o_sb[:], in1=b2_t[:],
                                        op=ALU.add)
                mx = sb.tile([P, 1], f32, tag="mx")
                nc.vector.tensor_reduce(out=mx[:], in_=o_sb[:, :OUTC],
                                        axis=mybir.AxisListType.X, op=ALU.max)
                t2 = t2_all[:, t * OUTC:(t + 1) * OUTC]
                nc.vector.tensor_tensor(out=t2, in0=o_sb[:, :OUTC],
                                        in1=mx[:].broadcast_to([P, OUTC]),
                                        op=ALU.subtract)
                exl = sb.tile([P, OUTC], f32, tag="exl")
                nc.scalar.activation(exl[:], t2, AF.Exp)
                nc.vector.tensor_reduce(out=sm_all[:, t:t + 1], in_=exl[:],
                                        axis=mybir.AxisListType.X, op=ALU.add)

            def agg_layer(l, on_tile_done):
                """Edge aggregation for layer l; on_tile_done(t) is called
                right after tile t is flushed (used to interleave the next
                layer's node phase + chunked allgather).

                Chunks are processed in two phases (early = slab-0-only
                sources for every tile first, then the rest) so the gather
                stream keeps running while the second allgather chunk is in
                flight; early partial sums are staged to DRAM/SBUF."""
                hwid = CO if l == 2 else HD
                colw = COL2 if l == 2 else COL0
                nhh = 1 if l == 2 else NH
                asrc_c = hwid
                sched = [(t, c) for t in range(bpc)
                         for c in range(nearly[t])] + \
                        [(t, c) for t in range(bpc)
                         for c in range(nearly[t], cpt)]
                state = {}
                pending = []
                for s in range(nsup):
                    g_s = min(G, ch - s * G)
                    c0 = s * G
                    # one-hot pm[e, slot-major (j,g)] (bf16); all operands
                    # keep a packed 2-byte last dim -> DVE 2x
                    pm = sb.tile([P, P * G], bf16, tag="pm")
                    pmv = pm[:].rearrange("p (j g) -> p j g", g=G)
                    nc.vector.tensor_tensor(
                        out=pmv[:, :, :g_s],
                        in0=dstf_t[:, c0:c0 + g_s]
                            .rearrange("p (one g) -> p one g", one=1)
                            .broadcast_to([P, P, g_s]),
                        in1=iota_t[:].rearrange("p (j g) -> p j g",
                                                g=G)[:, :, :g_s],
                        op=ALU.is_equal,
                    )

                    def pmc(g):  # [e, slot] view of chunk g (stride G)
                        return pm[:].rearrange("p (j g) -> p g j",
                                               g=G)[:, g:g + 1, :] \
                            .rearrange("p one j -> p (one j)")

                    # gather the source rows (one SWDGE call per chunk);
                    # early chunks only touch table slab 0, so they can run
                    # while the second allgather chunk is still in flight
                    rowg = sb.tile([P, G * colw], bf16, tag="rowg", bufs=5)
                    for g in range(g_s):
                        chk = c0 + g
                        t, c = sched[chk]
                        tbl = hcat[l][0:slab0, :] if c < nearly[t] \
                            else hcat[l][:, :]
                        nc.gpsimd.indirect_dma_start(
                            out=rowg[:, g * colw:(g + 1) * colw],
                            out_offset=None,
                            in_=tbl,
                            in_offset=bass.IndirectOffsetOnAxis(
                                ap=idx_t[:, chk:chk + 1], axis=0),
                        )
                    # transposed one-hot for the a_dst lookup
                    pt_ps = ps.tile([P, G * P], bf16, tag="tr", bufs=1)
                    for g in range(g_s):
                        nc.tensor.transpose(out=pt_ps[:, g * P:(g + 1) * P],
                                            in_=pmc(g),
                                            identity=ident_t[:])
                    pt_sb = sb.tile([P, G * P], bf16, tag="pt")
                    nc.scalar.activation(pt_sb[:, :g_s * P],
                                         pt_ps[:, :g_s * P], AF.Copy)
                    ade_ps = ps.tile([P, G * NH], f32, tag="ade", bufs=1)
                    for g in range(g_s):
                        chk = c0 + g
                        t, c = sched[chk]
                        if c == 0 or c == nearly[t]:
                            o_ps = ps.tile([P, hwid], f32, tag="agg",
                                           name="o_ps", bufs=4)
                            d_ps = ps.tile([P, nhh], f32, tag="den",
                                           name="d_ps", bufs=2)
                            state[t] = (o_ps, d_ps)
                        if l == 2:
                            adr = aad2_sb[:, 2 * t + 1:2 * t + 2]
                        else:
                            adr = aad_sb[l][:, t * 2 * NH + NH:
                                            (t + 1) * 2 * NH]
                        nc.tensor.matmul(
                            out=ade_ps[:, g * nhh:(g + 1) * nhh],
                            lhsT=pt_sb[:, g * P:(g + 1) * P],
                            rhs=adr, start=True, stop=True)

                    # attention math for the whole super
                    ne = g_s * nhh
                    rview = rowg[:, :g_s * colw].rearrange(
                        "p (g w) -> p g w", g=g_s)
                    ex = sb.tile([P, G * NH], f32, tag="ex")
                    tmp = sb.tile([P, G * NH], f32, tag="tmp")
                    nc.vector.tensor_tensor(
                        out=ex[:, :ne].rearrange("p (g h) -> p g h", g=g_s),
                        in0=rview[:, :, asrc_c:asrc_c + nhh],
                        in1=ade_ps[:, :ne].rearrange("p (g h) -> p g h",
                                                     g=g_s),
                        op=ALU.add)
                    nc.vector.tensor_scalar_mul(tmp[:, :ne], ex[:, :ne], NEG)
                    nc.vector.tensor_tensor(out=ex[:, :ne], in0=ex[:, :ne],
                                            in1=tmp[:, :ne], op=ALU.max)
                    exd = sb.tile([P, G * NH], bf16, tag="exd")
                    nc.scalar.activation(exd[:, :ne], ex[:, :ne], AF.Exp)
                    # h columns are (c,h)-interleaved so the per-head scale
                    # broadcast lands on a middle dim (keeps 2x mode)
                    sc = sb.tile([P, G * HD], bf16, tag="sc")
                    if l == 2:
                        nc.vector.tensor_tensor(
                            out=sc[:, :g_s * hwid].rearrange(
                                "p (g c) -> p g c", g=g_s),
                            in0=rview[:, :, :hwid],
                            in1=exd[:, :ne].rearrange(
                                "p (g one) -> p g one", g=g_s)
                                .broadcast_to([P, g_s, hwid]),
                            op=ALU.mult)
                    else:
                        nc.vector.tensor_tensor(
                            out=sc[:, :g_s * hwid].rearrange(
                                "p (g c h) -> p g c h", g=g_s, h=NH),
                            in0=rview[:, :, :hwid].rearrange(
                                "p g (c h) -> p g c h", h=NH),
                            in1=exd[:, :ne].rearrange(
                                "p (g one h) -> p g one h", one=1, h=NH)
                                .broadcast_to([P, g_s, HC, NH]),
                            op=ALU.mult)

                    # drain flushes deferred from the previous super so
                    # their DVE chains sit behind this super's bulk DVE ops
                    # (keeps the gather queue from head-of-line blocking)
                    for (pt_, po, pd) in pending:
                        if nearly[pt_] > 0:
                            hp = sb.tile([P, hwid], bf16, tag="hpart")
                            nc.sync.dma_start(
                                hp[:], opart[pt_ * P:(pt_ + 1) * P, :hwid])
                            dsl = dpart[:, pt_ * NH:pt_ * NH + nhh]
                        else:
                            hp, dsl = None, None
                        on_tile_done(pt_, po, pd, hp, dsl)
                    pending.clear()

                    # scatter matmuls + partial/final flushes
                    for g in range(g_s):
                        chk = c0 + g
                        t, c = sched[chk]
                        o_ps, d_ps = state[t]
                        st = (c == 0 or c == nearly[t])
                        sp = (c == nearly[t] - 1 or c == cpt - 1)
                        nc.tensor.matmul(
                            out=o_ps[:],
                            lhsT=pmc(g),
                            rhs=sc[:, g * hwid:(g + 1) * hwid],
                            start=st, stop=sp)
                        nc.tensor.matmul(
                            out=d_ps[:], lhsT=pmc(g),
                            rhs=exd[:, g * nhh:(g + 1) * nhh],
                            start=st, stop=sp)
                        if c == nearly[t] - 1 and c != cpt - 1:
                            # early partial: stage to DRAM/SBUF
                            part = sb.tile([P, hwid], bf16, tag="part")
                            nc.scalar.activation(part[:], o_ps[:], AF.Copy)
                            nc.sync.dma_start(
                                opart[t * P:(t + 1) * P, :hwid], part[:])
                            nc.vector.tensor_copy(
                                dpart[:, t * NH:t * NH + nhh], d_ps[:])
                            del state[t]
                        elif c == cpt - 1:
                            pending.append((t, o_ps, d_ps))
                            del state[t]
                for (pt_, po, pd) in pending:
                    if nearly[pt_] > 0:
                        hp = sb.tile([P, hwid], bf16, tag="hpart")
                        nc.sync.dma_start(
                            hp[:], opart[pt_ * P:(pt_ + 1) * P, :hwid])
                        dsl = dpart[:, pt_ * NH:pt_ * NH + nhh]
                    else:
                        hp, dsl = None, None
                    on_tile_done(pt_, po, pd, hp, dsl)
                pending.clear()

            # ---------- the program
            def finish_out(t0, t1):
                """log-softmax + output DMA for tiles [t0, t1)."""
                nt = t1 - t0
                ls = sb.tile([P, bpc], f32, tag="ls")
                nc.scalar.activation(ls[:, :nt], sm_all[:, t0:t1], AF.Ln)
                tv = t2_all[:, t0 * OUTC:t1 * OUTC]
                nc.vector.tensor_tensor(
                    out=tv.rearrange("p (t c) -> p t c", c=OUTC),
                    in0=tv.rearrange("p (t c) -> p t c", c=OUTC),
                    in1=ls[:, :nt].rearrange("p (t one) -> p t one", one=1)
                        .broadcast_to([P, nt, OUTC]),
                    op=ALU.subtract)
                nc.sync.dma_start(
                    out_ap[t0 * P:t1 * P, :]
                    .rearrange("(t p) c -> p t c", p=P),
                    tv.rearrange("p (t c) -> p t c", c=OUTC))

            def node_and_ag(l):
                done = [0]

                def cb(t, o_ps, d_ps, hp, dsl):
                    if l == 3:
                        flush_l2(t, o_ps, d_ps, hp, dsl)
                        done[0] += 1
                        qs = [bpc // 3, 2 * bpc // 3, bpc - 5, bpc]
                        if done[0] in qs:
                            i = qs.index(done[0])
                            finish_out(0 if i == 0 else qs[i - 1], qs[i])
                        return
                    flush_big(l - 1, t, o_ps, d_ps,
                              b0_t if l == 1 else b1_t, hp, dsl)
                    node_tile(l, t)
                    done[0] += 1
                    for j, (t0, t1) in enumerate(grp):
                        if done[0] == t1:
                            allgather_group(l, j)
                return cb

            # layer 0 node phase + chunked AG (no preceding agg).
            # In single mode the group-0 stand-in copy is drip-fed behind
            # the cat4 writes so the first gather isn't gated on one big
            # copy at the end of the node phase.
            wm0 = 0
            for t in range(bpc):
                node_tile(0, t)
                if single and (t + 1) in (8, 16) and (t + 1) <= grp[0][1]:
                    nc.sync.dma_start(hcat[0][wm0:(t + 1) * P, :],
                                      ag_in[0][wm0:(t + 1) * P, :])
                    wm0 = (t + 1) * P
                for j, (t0, t1) in enumerate(grp):
                    if t + 1 == t1:
                        if single and j == 0:
                            if wm0 < t1 * P:
                                nc.sync.dma_start(hcat[0][wm0:t1 * P, :],
                                                  ag_in[0][wm0:t1 * P, :])
                        else:
                            allgather_group(0, j)
                        if j == 0:
                            load_late_consts()
            agg_layer(0, node_and_ag(1))
            agg_layer(1, node_and_ag(2))
            agg_layer(2, node_and_ag(3))


    nc.compile()
    return nc


# ------------------------------------------------------------------ runners

_CACHE = {}


def _get_program(dims):
    key = tuple(sorted(dims.items()))
    if key not in _CACHE:
        try:
            _CACHE[key] = _build(dims, shared_ag=True)
        except Exception:
            _CACHE[key] = _build(dims, shared_ag=False)
    return _CACHE[key]


def make_in_maps(x, W0, as0, ad0, b0, W1, as1, ad1, b1, W2, as2, ad2, b2,
                 dims, per_core, new_id):
    npad, nloc = dims["npad"], dims["nloc"]
    xp = np.zeros((npad, D0), np.float32)
    xp[new_id] = np.asarray(x, np.float32)

    a0 = _block_diag_a(np.asarray(as0, np.float32),
                       np.asarray(ad0, np.float32))
    a1 = _block_diag_a(np.asarray(as1, np.float32),
                       np.asarray(ad1, np.float32))
    W0f = np.asarray(W0, np.float32)
    W1f = np.asarray(W1, np.float32)
    W2f = np.asarray(W2, np.float32)
    w2e = np.zeros((HD, COL2 + 1), np.float32)
    w2e[:, :OUTC] = W2f
    w2e[:, CO] = W2f @ np.asarray(as2, np.float32)[0]
    w2e[:, CO + 1] = W2f @ np.asarray(ad2, np.float32)[0]
    b2p = np.zeros((1, CO), np.float32)
    b2p[0, :OUTC] = b2

    # (c,h)-interleaved feature permutation: table col c*NH+h <- feature
    # h*HC+c.  Applied to table columns and matching weight rows.
    fi = np.arange(HD)
    perm = (fi % NH) * HC + fi // NH

    # iotag[p, j*G+g] = j  (slot-major one-hot comparand)
    iotag = np.repeat(np.arange(P, dtype=np.float32), G)[None, :] \
        .repeat(P, axis=0)

    shared = {
        "w0h": W0f[:, perm].astype(BF),
        "w0a": (W0f @ a0).astype(BF),
        "w1h": W1f[perm][:, perm].astype(BF),
        "w1a": (W1f @ a1)[perm].astype(BF),
        "w2e": w2e[perm].astype(BF),
        "b0r": np.asarray(b0, np.float32).reshape(1, HD)[:, perm],
        "b1r": np.asarray(b1, np.float32).reshape(1, HD)[:, perm],
        "b2r": b2p,
        "iotag": iotag.astype(BF),
        "identb": np.eye(P, dtype=np.float32).astype(BF),
    }
    in_maps = []
    for c in range(NCORES):
        m = dict(shared)
        m["xt"] = np.ascontiguousarray(
            xp[c * nloc:(c + 1) * nloc].T).astype(BF)
        m.update(per_core[c])
        in_maps.append(m)
    return in_maps


def assemble_output(results, dims, new_id):
    n = dims["n"]
    full = np.concatenate([results[c]["out"] for c in range(NCORES)], axis=0)
    return np.ascontiguousarray(full[new_id[:n]])


def kernel(x, edge_index, W0, as0, ad0, b0, W1, as1, ad1, b1,
           W2, as2, ad2, b2):
    from concourse import bass_utils

    n = x.shape[0]
    dims, per_core, new_id = _prep(np.asarray(edge_index), n)
    prog = _get_program(dims)
    in_maps = make_in_maps(x, W0, as0, ad0, b0, W1, as1, ad1, b1,
                           W2, as2, ad2, b2, dims, per_core, new_id)
    res = bass_utils.run_bass_kernel_spmd(prog, in_maps,
                                          core_ids=list(range(NCORES)))
    return assemble_output(res.results, dims, new_id)
